# revision 12
# baseline (speedup 1.0000x reference)
"""Trainium2 Bass kernel for nn_ConditionalEstimation (v5: bf16 stream).

Computes, for full inputs:
    context[i] = sum_{j,k} a[i,j,k] * y[j] * z[k]          (i in [0, 384))
    scores[n]  = (x[n, :] @ context) / (context[0] + 1e-8)

Sharding across 8 NeuronCores (SPMD, one NEFF):
    - a is sharded along its leading i axis: core c owns a[c*48:(c+1)*48].
      Each core computes its 48-element slice of `context`, then an
      AllGather assembles the full 384-vector on every core.
    - x_candidates is sharded along N: core c owns rows [c*8192, (c+1)*8192)
      and computes those scores (pure data parallel).

v5 changes vs v4 (171us baseline):
    - a and x are cast to bf16 on the host: the kernel is HBM-bound and
      this halves the stream (20.5 MB/core vs 41 MB/core). fp32 accum in
      PSUM keeps the error ~2.6e-3 against the fp32 reference.
    - host pre-permutes a and x so every DMA is one contiguous run per
      partition (6.9KB/3KB lines) -> line-rate descriptors.
    - all a/x buffers are fully resident in SBUF (no pool backpressure):
      a on sync+scalar queues first, x behind them on sync, so the a
      stream (which gates the AllGather) finishes first at full rate.
    - phase-1 j-contraction PE matmuls write tile t's three group rows to
      PSUM partitions {0, 32, 64} (the only legal matmul output bases) of
      one [65, 384] bank; the z-finisher is then one DVE mul + one
      reduce per tile instead of 6 serial partition-0 ops — 32 DVE ops
      total instead of 96. (tensor_tensor_reduce would fuse these but
      crashes this runtime at execute time.)
    - phase-2 keeps per-chunk [1, 512] PSUM rows; all chunk outputs
      collect in one SBUF row and leave in a single DMA.
"""

import os
import sys

import numpy as np

sys.path.insert(0, "/opt/trn_rl_repo")

import ml_dtypes

import concourse.bacc as bacc
import concourse.mybir as mybir
import concourse.tile as tile
from concourse.bass_utils import run_bass_kernel_spmd

N, D = 65536, 384
NC = 8
ISH = D // NC            # 48 context rows per core
XSH = N // NC            # 8192 candidate rows per core
EPS = 1e-8
FP = mybir.dt.float32
BF = mybir.dt.bfloat16

NT = 16                  # phase-1 a tiles (3 i-groups each)
GPT = ISH // NT          # groups per tile (3)

PCH = 512                # PE path chunk width
NCH = XSH // PCH         # PE path chunks (16)

_CACHE = {}
LAST_RESULT = None  # BassKernelResults of the most recent run (for test harness)


def _build():
    if "nc" in _CACHE:
        return _CACHE["nc"]

    nc = bacc.Bacc("TRN2", target_bir_lowering=False, debug=False, num_devices=NC)
    Alu = mybir.AluOpType
    Act = mybir.ActivationFunctionType

    # a tile t, partition p, group g, sub-row s, col k = a_sh[3t+g, 3p+s, k]
    a_d = nc.dram_tensor("a_sh", [NT, 128, GPT, 3, D], BF, kind="ExternalInput")
    # x chunk c, partition p, s, q = x_sh[512c + q, 128s + p]
    xp_d = nc.dram_tensor("xT_pe", [NCH, 128, 3, PCH], BF, kind="ExternalInput")
    y3_d = nc.dram_tensor("y3p", [128, 3], BF, kind="ExternalInput")
    zb_d = nc.dram_tensor("z_b", [65, D], FP, kind="ExternalInput")
    o_d = nc.dram_tensor("scores_sh", [XSH], FP, kind="ExternalOutput")

    with tile.TileContext(nc) as tc:
        with (
            tc.tile_pool(name="const", bufs=1) as cst,
            tc.tile_pool(name="a", bufs=NT) as a_pool,
            tc.tile_pool(name="xtp", bufs=NCH) as xt_pool,
            tc.tile_pool(name="scr", bufs=2) as scr_pool,
            tc.tile_pool(name="acc", bufs=1) as acc_pool,
            tc.tile_pool(name="psA", bufs=2, space="PSUM") as psA,
            tc.tile_pool(name="psW", bufs=2, space="PSUM") as psW,
            tc.tile_pool(name="psT", bufs=1, space="PSUM") as psT,
            tc.tile_pool(name="psS", bufs=3, space="PSUM") as psS,
            tc.tile_pool(name="dram", bufs=1, space="DRAM") as dram_pool,
        ):
            # --- warm-up collective: absorbs the CC-stream entry barrier +
            # ncfw startup + cross-core launch skew while the a/x stream
            # runs. Triggered as early as possible.
            dummy = cst.tile([1, 8], FP)
            nc.vector.memset(dummy[:], 0.0)
            cc_din = dram_pool.tile([8], FP)
            cc_dout = dram_pool.tile([8 * NC], FP)
            nc.scalar.dma_start(cc_din[:], dummy[:])
            nc.gpsimd.collective_compute(
                "AllGather",
                Alu.bypass,
                replica_groups=[list(range(NC))],
                ins=[cc_din.opt()],
                outs=[cc_dout.opt()],
            )

            # --- constants (scalar HWDGE queue, ahead of its a tiles) ---
            y3p = cst.tile([128, 3], BF)
            nc.scalar.dma_start(y3p[:], y3_d.ap())
            zb = cst.tile([65, D], FP)
            nc.scalar.dma_start(zb[:], zb_d.ap())
            ones11 = cst.tile([1, 1], FP)
            nc.vector.memset(ones11[:], 1.0)

            # --- PE preheat: ramp the p-state before the first a tile ---
            for w in range(12):
                wps = psW.tile([1, 3], FP, tag="wk")
                nc.tensor.matmul(
                    wps[:], y3p[:, 0:1], y3p[:], start=True, stop=True,
                )

            # --- a stream: 8 tiles on sync, 8 on scalar, fully resident ---
            ats = []
            for t in range(NT):
                at = a_pool.tile([128, GPT, 3, D], BF, tag="a")
                eng = nc.sync if t % 2 == 0 else nc.scalar
                eng.dma_start(at[:], a_d.ap()[t])
                ats.append(at)

            # --- x stream: all 16 chunks on sync, behind the a tiles ---
            xcs = []
            for c in range(NCH):
                xc = xt_pool.tile([128, 3, PCH], BF)
                nc.sync.dma_start(xc[:], xp_d.ap()[c])
                xcs.append(xc)

            # --- phase 1: j-contraction; tile t's group g2 -> PSUM base
            # partition 32*g2 of a [65, 384] bank, then ONE fused DVE
            # mul(z)+reduce per tile -> accumulator column scrA[:, t].
            # Lanes 1-31 / 33-63 of the [65, x] ops carry garbage that is
            # never read back.
            scrA = acc_pool.tile([65, NT], FP)
            for t in range(NT):
                at = ats[t]
                ups3 = psA.tile([65, D], FP, tag="ups")
                for g2 in range(GPT):
                    for s in range(3):
                        nc.tensor.matmul(
                            ups3[32 * g2:32 * g2 + 1, :],
                            y3p[:, s:s + 1], at[:, g2, s, :],
                            start=(s == 0), stop=(s == 2),
                        )
                scrO = scr_pool.tile([65, D], FP, tag="fin")
                nc.vector.tensor_mul(scrO[:], ups3[:], zb[:])
                nc.vector.tensor_reduce(
                    scrA[:, t:t + 1], scrO[:],
                    axis=mybir.AxisListType.X, op=Alu.add,
                )

            # --- AllGather the context slices ---
            # cc_in[i_local] with i_local = 3t + g2 lives at scrA[32*g2, t]
            cc_in = dram_pool.tile([ISH], FP)
            cc_out = dram_pool.tile([D], FP)
            for g2 in range(GPT):
                nc.scalar.dma_start(
                    cc_in[:].rearrange("(t g) -> g t", g=GPT)[g2],
                    scrA[32 * g2:32 * g2 + 1, :],
                )
            nc.gpsimd.collective_compute(
                "AllGather",
                Alu.bypass,
                replica_groups=[list(range(NC))],
                ins=[cc_in.opt()],
                outs=[cc_out.opt()],
            )

            # --- PE warm-keepers: dependency-free matmuls keep the PE clock
            # high through the AllGather window (~12us << AG latency).
            for w in range(20):
                wps = psW.tile([1, D], FP, tag="wk")
                for s2 in range(2):
                    nc.tensor.matmul(
                        wps[:], y3p[:, s2:s2 + 1], ats[-1][:, 0, s2, :],
                        start=(s2 == 0), stop=(s2 == 1),
                    )

            # --- post-AG setup ---
            ctxrow = cst.tile([1, D], FP)
            nc.scalar.dma_start(ctxrow[:], cc_out[:].unsqueeze(0))
            rec1 = cst.tile([1, 1], FP)
            nc.vector.tensor_scalar_add(rec1[:], ctxrow[:, 0:1], EPS)
            nc.vector.reciprocal(rec1[:], rec1[:])
            ctxT = psT.tile([128, 3], FP)
            for s2 in range(3):
                nc.tensor.matmul(
                    ctxT[:, s2:s2 + 1], ctxrow[:, 128 * s2:128 * (s2 + 1)],
                    ones11[:], start=True, stop=True,
                )
            ctx3b = cst.tile([128, 3], BF)   # bf16 stationary for phase 2
            nc.vector.tensor_scalar_mul(ctx3b[:], ctxT[:], 1.0)

            # --- phase 2: per-chunk matvec; the 1/(den+eps) scale folds
            # into the PSUM->SBUF copies; one output DMA at the end.
            so_all = acc_pool.tile([1, XSH], FP)
            for c in range(NCH):
                sps = psS.tile([1, PCH], FP, tag="ps")
                for s in range(3):
                    nc.tensor.matmul(
                        sps[:], ctx3b[:, s:s + 1], xcs[c][:, s, :],
                        start=(s == 0), stop=(s == 2),
                    )
                dst = so_all[:, c * PCH:(c + 1) * PCH]
                nc.scalar.activation(dst, sps[:], Act.Copy, scale=rec1[:])
            nc.sync.dma_start(o_d.ap(), so_all[:])

    nc.compile()
    _CACHE["nc"] = nc
    return nc


def make_in_maps(x_candidates, y, z, a):
    x_candidates = np.ascontiguousarray(x_candidates, dtype=np.float32)
    y = np.ascontiguousarray(y, dtype=np.float32)
    z = np.ascontiguousarray(z, dtype=np.float32)
    a = np.ascontiguousarray(a, dtype=np.float32)
    bf = ml_dtypes.bfloat16
    y3p = np.ascontiguousarray(y.reshape(128, 3).astype(bf))
    z_b = np.ascontiguousarray(np.broadcast_to(z, (65, D)))
    in_maps = []
    for c in range(NC):
        # x chunk-major: [NCH, 128, 3, PCH]; [c, p, s, q] = x[512c+q, 128s+p]
        x_sh = x_candidates[c * XSH:(c + 1) * XSH]
        xt = np.ascontiguousarray(
            x_sh.T.reshape(3, 128, NCH, PCH).transpose(2, 1, 0, 3).astype(bf)
        )
        # a tiles: [NT, 128, GPT, 3, D]; [t, p, g, s, k] = a_sh[3t+g, 3p+s, k]
        a_sh = a[c * ISH:(c + 1) * ISH].reshape(NT, GPT, 128, 3, D)
        a_t = np.ascontiguousarray(a_sh.transpose(0, 2, 1, 3, 4).astype(bf))
        in_maps.append({
            "a_sh": a_t,
            "xT_pe": xt,
            "y3p": y3p,
            "z_b": z_b,
        })
    return in_maps


def kernel(x_candidates, y, z, a):
    global LAST_RESULT
    nc = _build()
    in_maps = make_in_maps(x_candidates, y, z, a)

    trace = os.environ.get("CC_KERNEL_TRACE", "0") == "1"
    try:
        res = run_bass_kernel_spmd(nc, in_maps, core_ids=list(range(NC)), trace=trace)
    except Exception:
        if not trace:
            raise
        # Trace post-processing can fail in minimal containers; results
        # are what matter — retry without tracing.
        res = run_bass_kernel_spmd(nc, in_maps, core_ids=list(range(NC)), trace=False)
    LAST_RESULT = res
    out = np.concatenate([res.results[c]["scores_sh"] for c in range(NC)])
    return np.ascontiguousarray(out, dtype=np.float32)
